# revision 10
# baseline (speedup 1.0000x reference)
import sys
sys.path.insert(0, '/opt/trn_rl_repo')
import numpy as np
import concourse.bass as bass
import concourse.bacc as bacc
import concourse.mybir as mybir
from concourse.tile import TileContext
from concourse.bass_utils import run_bass_kernel_spmd

F32 = mybir.dt.float32
F32R = mybir.dt.float32r
BF16 = mybir.dt.bfloat16
AF = mybir.ActivationFunctionType
ALU = mybir.AluOpType
EPS = 1e-5

B, C, D, HH, W = 2, 256, 32, 32, 32
S = D * HH * W            # 32768 spatial per batch
NCORES = 8
SHARDS = 4                # spatial shards per batch
T_TOT = S // SHARDS       # 8192 tokens per core
CHUNK = 512
NCH = T_TOT // CHUNK      # 16 chunks
PIECE = 4096              # x load piece (cols)
NPIECE = T_TOT // PIECE   # 2
CTX, CD = 77, 768
H, HD = 8, 32
SCALE = HD ** -0.5
GN_N = float(32 * S)      # elements per group (32 ch x full spatial)

# packed param columns
PC_GG = 0      # 2 cols (per channel-half)
PC_GB = 2      # 2
PC_LG = 4      # 6
PC_LB = 10     # 6
PC_BD = 16     # 128 (32-block-diagonal ones)
PC_ID = 144    # 128 (identity)
PC_BM = 272    # 2 (batch-mask: col b = 1.0 if my batch == b)
NPAR = 274

_cache = {}


def _build():
    nc = bacc.Bacc("TRN2", target_bir_lowering=False, debug=False, num_devices=NCORES)
    dp = lambda n, s: nc.dram_tensor(n, s, F32, kind="ExternalInput").ap()
    x_d = dp("x", [C, T_TOT])
    ctx_d = dp("ctx", [CTX, CD])
    wkv_d = dp("wkv", [CD, 512])   # [Wk | Wv]
    wqo_d = dp("wqo", [C, 512])    # [Wq | Wo]
    par_d = dp("par", [128, NPAR])
    out_d = nc.dram_tensor("out", [C, T_TOT], F32, kind="ExternalOutput").ap()
    gn_in = nc.dram_tensor("gn_in", [128, 4], F32)
    gn_out = nc.dram_tensor("gn_out", [1024, 4], F32, addr_space="Shared")

    with TileContext(nc, num_cores=NCORES) as tc:
        with tc.tile_pool(name="const", bufs=1) as const, \
             tc.tile_pool(name="work", bufs=2) as work, \
             tc.tile_pool(name="pqt", bufs=1, space="PSUM") as p_qt, \
             tc.tile_pool(name="pscor", bufs=4, space="PSUM") as p_scor, \
             tc.tile_pool(name="pattn", bufs=1, space="PSUM") as p_attn, \
             tc.tile_pool(name="pzo", bufs=2, space="PSUM") as p_zo:

            # ---------- loads: x pieces first, then params/weights ----------
            xs = [[const.tile([128, PIECE], F32R, tag=f"x{i}{p}", name=f"x{i}{p}")
                   for p in range(NPIECE)] for i in range(2)]
            for p in range(NPIECE):
                for i in range(2):
                    nc.sync.dma_start(
                        out=xs[i][p][:],
                        in_=x_d[i*128:(i+1)*128, p*PIECE:(p+1)*PIECE].bitcast(F32R))
            par_sb = const.tile([128, NPAR], F32)
            nc.sync.dma_start(out=par_sb[:], in_=par_d[:])
            ctx_sb = const.tile([CTX, CD], F32)
            nc.sync.dma_start(out=ctx_sb[:], in_=ctx_d[:])
            wkv_sb = [const.tile([128, 512], F32, tag=f"wkv{k}", name=f"wkv{k}")
                      for k in range(6)]
            for k in range(6):
                nc.sync.dma_start(out=wkv_sb[k][:], in_=wkv_d[k*128:(k+1)*128, :])
            wqo_sb = [const.tile([128, 512], F32R, tag=f"wqo{i}", name=f"wqo{i}")
                      for i in range(2)]
            for i in range(2):
                nc.sync.dma_start(out=wqo_sb[i][:],
                                  in_=wqo_d[i*128:(i+1)*128, :].bitcast(F32R))

            # ---------- GroupNorm partial stats (pipelined with x arrival) ----------
            st = [work.tile([128, 2*PIECE//512, 6], F32, tag=f"st{i}", name=f"st{i}")
                  for i in range(2)]
            for p in range(NPIECE):
                for i in range(2):
                    for c in range(PIECE // 512):
                        j = p * (PIECE // 512) + c
                        nc.vector.bn_stats(out=st[i][:, j, :],
                                           in_=xs[i][p][:, c*512:(c+1)*512].bitcast(F32))
            partials = work.tile([128, 4], F32, tag="partials", name="partials")
            for i in range(2):
                mv = work.tile([128, 2], F32, tag="gnmv", name=f"gnmv{i}")
                nc.vector.bn_aggr(out=mv[:], in_=st[i][:])
                sq = work.tile([128, 1], F32, tag="gnsq", name=f"gnsq{i}")
                nc.vector.tensor_mul(sq[:], mv[:, 0:1], mv[:, 0:1])
                nc.vector.tensor_add(sq[:], sq[:], mv[:, 1:2])
                nc.vector.tensor_scalar_mul(partials[:, 2*i:2*i+1], mv[:, 0:1], float(T_TOT))
                nc.vector.tensor_scalar_mul(partials[:, 2*i+1:2*i+2], sq[:], float(T_TOT))
            nc.sync.dma_start(out=gn_in[:], in_=partials[:])
            nc.gpsimd.collective_compute(
                "AllGather", ALU.bypass,
                replica_groups=[[0, 1, 2, 3, 4, 5, 6, 7]],
                ins=[gn_in[:]], outs=[gn_out[:]])
            gw = work.tile([128, 8, 4], F32, tag="gw", name="gw")
            nc.sync.dma_start(out=gw[:],
                              in_=gn_out[:].rearrange("(b r) c -> r b c", b=8))
            u = work.tile([128, 2, 4], F32, tag="gu", name="gu")
            nc.vector.tensor_add(u[:], gw[:, 0:2, :], gw[:, 2:4, :])   # batch0 blocks
            v = work.tile([128, 2, 4], F32, tag="gv", name="gv")
            nc.vector.tensor_add(v[:], gw[:, 4:6, :], gw[:, 6:8, :])   # batch1 blocks
            nc.vector.tensor_add(u[:, 0, :], u[:, 0, :], u[:, 1, :])
            nc.vector.tensor_add(v[:, 0, :], v[:, 0, :], v[:, 1, :])
            nc.vector.tensor_scalar_mul(u[:, 0, :], u[:, 0, :], par_sb[:, PC_BM:PC_BM+1])
            nc.vector.tensor_scalar_mul(v[:, 0, :], v[:, 0, :], par_sb[:, PC_BM+1:PC_BM+2])
            nc.vector.tensor_add(u[:, 0, :], u[:, 0, :], v[:, 0, :])
            gps = p_zo.tile([128, 4], F32, tag="zo", name="gps")
            nc.tensor.matmul(gps[:], par_sb[:, PC_BD:PC_BD+128], u[:, 0, :],
                             start=True, stop=True)

            eps_t = work.tile([128, 1], F32, tag="eps", name="eps")
            nc.vector.memset(eps_t[:], EPS)
            a_sb, b_sb = [], []
            for i in range(2):
                mu = work.tile([128, 1], F32, tag=f"mu{i}", name=f"mu{i}")
                nc.vector.tensor_scalar_mul(mu[:], gps[:, 2*i:2*i+1], 1.0 / GN_N)
                ms = work.tile([128, 1], F32, tag=f"ms{i}", name=f"ms{i}")
                nc.vector.tensor_scalar_mul(ms[:], gps[:, 2*i+1:2*i+2], 1.0 / GN_N)
                var = work.tile([128, 1], F32, tag=f"var{i}", name=f"var{i}")
                nc.vector.tensor_mul(var[:], mu[:], mu[:])
                nc.vector.tensor_sub(var[:], ms[:], var[:])
                std = work.tile([128, 1], F32, tag=f"std{i}", name=f"std{i}")
                nc.scalar.activation(out=std[:], in_=var[:], func=AF.Sqrt, bias=eps_t[:])
                rstd = work.tile([128, 1], F32, tag=f"rstd{i}", name=f"rstd{i}")
                nc.vector.reciprocal(out=rstd[:], in_=std[:])
                ai = const.tile([128, 1], F32, tag=f"ai{i}", name=f"ai{i}")
                nc.vector.tensor_mul(ai[:], rstd[:], par_sb[:, PC_GG+i:PC_GG+i+1])
                bi = const.tile([128, 1], F32, tag=f"bi{i}", name=f"bi{i}")
                nc.vector.tensor_mul(bi[:], mu[:], ai[:])
                nc.vector.tensor_sub(bi[:], par_sb[:, PC_GB+i:PC_GB+i+1], bi[:])
                a_sb.append(ai)
                b_sb.append(bi)

            # ---------- context layernorm + K/V prep (overlaps x/stats) ----------
            stats = work.tile([CTX, 3, 6], F32, tag="lnstats", name="lnstats")
            for i in range(3):
                nc.vector.bn_stats(out=stats[:, i, :], in_=ctx_sb[:, i*256:(i+1)*256])
            mvc = work.tile([CTX, 2], F32, tag="lnmv", name="lnmv")
            nc.vector.bn_aggr(out=mvc[:], in_=stats[:])
            stdc = work.tile([CTX, 1], F32, tag="lnstd", name="lnstd")
            nc.scalar.activation(out=stdc[:], in_=mvc[:, 1:2], func=AF.Sqrt,
                                 bias=eps_t[:CTX, :])
            rstdc = work.tile([CTX, 1], F32, tag="lnrstd", name="lnrstd")
            nc.vector.reciprocal(out=rstdc[:], in_=stdc[:])
            ctxn = work.tile([CTX, CD], F32, tag="ctxn", name="ctxn")
            nc.vector.tensor_scalar(out=ctxn[:], in0=ctx_sb[:], scalar1=mvc[:, 0:1],
                                    scalar2=rstdc[:], op0=ALU.subtract, op1=ALU.mult)

            # transpose ctxn -> 6 tiles [128, 77] bf16, fold ln gamma/beta
            ctxnT = [const.tile([128, CTX], F32, tag=f"cT{k}", name=f"cT{k}")
                     for k in range(6)]
            for k in range(6):
                ps = p_zo.tile([128, CTX], F32, tag="zo", name=f"psT{k}")
                nc.tensor.transpose(ps[:, :], ctxn[:, k*128:(k+1)*128],
                                    par_sb[:CTX, PC_ID:PC_ID+CTX])
                nc.vector.tensor_scalar(out=ctxnT[k][:], in0=ps[:],
                                        scalar1=par_sb[:, PC_LG+k:PC_LG+k+1],
                                        scalar2=par_sb[:, PC_LB+k:PC_LB+k+1],
                                        op0=ALU.mult, op1=ALU.add)

            # kT [2 x (128, 77)] scaled by 1/sqrt(hd)
            kT = [const.tile([128, CTX], F32R, tag=f"kT{m}", name=f"kT{m}")
                  for m in range(2)]
            for m in range(2):
                ps = p_zo.tile([128, CTX], F32, tag="zo", name=f"psK{m}")
                for k in range(6):
                    nc.tensor.matmul(ps[:], wkv_sb[k][:, m*128:(m+1)*128],
                                     ctxnT[k][:], start=(k == 0), stop=(k == 5))
                nc.vector.tensor_scalar_mul(kT[m][:], ps[:], SCALE)

            # v [77, 256]
            v_sb = const.tile([CTX, C], F32)
            psv = p_attn.tile([CTX, C], F32, tag="attn", name="psV")
            for k in range(6):
                nc.tensor.matmul(psv[:], ctxnT[k][:], wkv_sb[k][:, 256:512],
                                 start=(k == 0), stop=(k == 5))
            nc.vector.tensor_copy(v_sb[:], psv[:])

            # zero-padded V / ones lhsT tiles [77, 128]
            zpad = const.tile([CTX, 128], F32)
            nc.gpsimd.memset(zpad[:], 0.0)
            onescol = const.tile([CTX, 32], F32)
            nc.gpsimd.memset(onescol[:], 1.0)
            vpad, opad = [], []
            for g in range(2):
                vp = [const.tile([CTX, 128], F32R, tag=f"vp{g}{j}", name=f"vp{g}{j}")
                      for j in range(4)]
                op = [const.tile([CTX, 128], F32R, tag=f"op{g}{j}", name=f"op{g}{j}")
                      for j in range(4)]
                for j in range(4):
                    h = 4*g + j
                    nc.gpsimd.tensor_copy(vp[j][:], zpad[:])
                    nc.gpsimd.tensor_copy(vp[j][:, 32*j:32*(j+1)], v_sb[:, 32*h:32*(h+1)])
                    nc.gpsimd.tensor_copy(op[j][:], zpad[:])
                    nc.gpsimd.tensor_copy(op[j][:, 32*j:32*(j+1)], onescol[:])
                vpad.append(vp)
                opad.append(op)

            # ---------- main token loop (software-pipelined) ----------
            state = {}  # per-(t-1) tiles carried across pipeline stages

            def emit_A(t):
                p, off = t // (PIECE // CHUNK), (t % (PIECE // CHUNK)) * CHUNK
                xn = []
                for i in range(2):
                    xni = work.tile([128, CHUNK], F32R, tag=f"xn{i}", name=f"xn{i}_{t}")
                    nc.gpsimd.tensor_scalar(out=xni[:],
                                            in0=xs[i][p][:, off:off+CHUNK].bitcast(F32),
                                            scalar1=a_sb[i][:], scalar2=b_sb[i][:],
                                            op0=ALU.mult, op1=ALU.add)
                    xn.append(xni)
                return {"t": t, "xn": xn, "p": p, "off": off}

            def emit_qt(A, m):
                ps = p_qt.tile([128, CHUNK], F32, tag="qt", name=f"psqt{m}_{A['t']}")
                for i in range(2):
                    nc.tensor.matmul(ps[:], wqo_sb[i][:, m*128:(m+1)*128],
                                     A["xn"][i][:], start=(i == 0), stop=(i == 1))
                qm = work.tile([128, CHUNK], F32R, tag=f"qT{m}", name=f"qT{m}_{A['t']}")
                nc.vector.tensor_copy(qm[:], ps[:])
                return qm

            def emit_scores(A, g, js, qm):
                t = A["t"]
                if "exps" not in A:
                    A["exps"] = {}
                if g not in A["exps"]:
                    A["exps"][g] = work.tile([CTX, 4*CHUNK], F32R, tag=f"exps{g}",
                                             name=f"exps{g}_{t}")
                ex = A["exps"][g]
                for j in js:
                    sl = p_scor.tile([CTX, CHUNK], F32, tag="scor", name=f"sc{g}{j}_{t}")
                    nc.tensor.matmul(sl[:], kT[g][32*j:32*(j+1), :],
                                     qm[32*j:32*(j+1), :],
                                     start=True, stop=True, tile_position=(32*j, 0))
                    nc.scalar.activation(out=ex[:, j*CHUNK:(j+1)*CHUNK], in_=sl[:],
                                         func=AF.Exp)

            def emit_attn(Bst, g):
                t = Bst["t"]
                ap = p_attn.tile([128, CHUNK], F32, tag="attn", name=f"attn{g}_{t}")
                for j in range(4):
                    nc.tensor.matmul(ap[:], vpad[g][j][:],
                                     Bst["exps"][g][:, j*CHUNK:(j+1)*CHUNK],
                                     start=(j == 0), stop=(j == 3))
                Bst[f"attn{g}"] = ap

            def emit_zb(Bst, g):
                t = Bst["t"]
                zp = p_zo.tile([128, CHUNK], F32, tag="zo", name=f"zb{g}_{t}")
                for j in range(4):
                    nc.tensor.matmul(zp[:], opad[g][j][:],
                                     Bst["exps"][g][:, j*CHUNK:(j+1)*CHUNK],
                                     start=(j == 0), stop=(j == 3))
                zr = work.tile([128, CHUNK], F32, tag=f"zr{g}", name=f"zr{g}_{t}")
                nc.vector.reciprocal(out=zr[:], in_=zp[:])
                Bst[f"zr{g}"] = zr

            def emit_anrm(Bst, g):
                an = work.tile([128, CHUNK], F32R, tag=f"anrm{g}", name=f"anrm{g}_{Bst['t']}")
                nc.vector.tensor_mul(an[:], Bst[f"attn{g}"][:], Bst[f"zr{g}"][:])
                Bst[f"anrm{g}"] = an

            def emit_out(Bst, m):
                t = Bst["t"]
                ps = p_zo.tile([128, CHUNK], F32, tag="zo", name=f"psout{m}_{t}")
                for g in range(2):
                    nc.tensor.matmul(ps[:], wqo_sb[g][:, 256+m*128:256+(m+1)*128],
                                     Bst[f"anrm{g}"][:], start=(g == 0), stop=(g == 1))
                o = work.tile([128, CHUNK], F32, tag=f"osb{m}", name=f"osb{m}_{t}")
                p, off = Bst["p"], Bst["off"]
                nc.vector.tensor_add(o[:], ps[:], xs[m][p][:, off:off+CHUNK].bitcast(F32))
                t0 = t * CHUNK
                nc.sync.dma_start(out=out_d[m*128:(m+1)*128, t0:t0+CHUNK], in_=o[:])

            prev = None
            for t in range(NCH):
                A = emit_A(t)
                q0m = emit_qt(A, 0)
                if prev is not None:
                    emit_attn(prev, 0)
                q1m = emit_qt(A, 1)
                if prev is not None:
                    emit_zb(prev, 0)
                emit_scores(A, 0, [0, 1], q0m)
                if prev is not None:
                    emit_anrm(prev, 0)
                    emit_attn(prev, 1)
                emit_scores(A, 0, [2, 3], q0m)
                if prev is not None:
                    emit_zb(prev, 1)
                emit_scores(A, 1, [0, 1, 2, 3], q1m)
                if prev is not None:
                    emit_anrm(prev, 1)
                    emit_out(prev, 0)
                    emit_out(prev, 1)
                prev = A
            # drain last chunk
            emit_attn(prev, 0)
            emit_zb(prev, 0)
            emit_anrm(prev, 0)
            emit_attn(prev, 1)
            emit_zb(prev, 1)
            emit_anrm(prev, 1)
            emit_out(prev, 0)
            emit_out(prev, 1)

    nc.compile()
    return nc


def _get_nc():
    if "nc" not in _cache:
        _cache["nc"] = _build()
    return _cache["nc"]


def kernel(x, context, gn_gamma, gn_beta, ln_gamma, ln_beta, Wq, Wk, Wv, Wo, bo,
           _trace=False):
    nc = _get_nc()
    x = np.asarray(x, dtype=np.float32)
    xr = x.reshape(B, C, S)
    par = np.zeros((128, NPAR), np.float32)
    par[:, PC_GG:PC_GG+2] = np.asarray(gn_gamma, np.float32).reshape(2, 128).T
    par[:, PC_GB:PC_GB+2] = np.asarray(gn_beta, np.float32).reshape(2, 128).T
    par[:, PC_LG:PC_LG+6] = np.asarray(ln_gamma, np.float32).reshape(6, 128).T
    par[:, PC_LB:PC_LB+6] = np.asarray(ln_beta, np.float32).reshape(6, 128).T
    par[:, PC_BD:PC_BD+128] = np.kron(np.eye(4, dtype=np.float32),
                                      np.ones((32, 32), np.float32))
    par[:, PC_ID:PC_ID+128] = np.eye(128, dtype=np.float32)
    common = {
        "wkv": np.ascontiguousarray(np.concatenate(
            [np.asarray(Wk, np.float32), np.asarray(Wv, np.float32)], axis=1)),
        "wqo": np.ascontiguousarray(np.concatenate(
            [np.asarray(Wq, np.float32), np.asarray(Wo, np.float32)], axis=1)),
        "par": par,
    }
    in_maps = []
    for core in range(NCORES):
        b, s = core // SHARDS, core % SHARDS
        m = dict(common)
        pc = par.copy()
        pc[:, PC_BM + b] = 1.0
        m["par"] = pc
        m["x"] = np.ascontiguousarray(xr[b][:, s*T_TOT:(s+1)*T_TOT])
        m["ctx"] = np.ascontiguousarray(np.asarray(context, np.float32)[b])
        in_maps.append(m)
    res = run_bass_kernel_spmd(nc, in_maps, list(range(NCORES)), trace=_trace)
    out = np.empty((B, C, S), np.float32)
    for core in range(NCORES):
        b, s = core // SHARDS, core % SHARDS
        out[b][:, s*T_TOT:(s+1)*T_TOT] = res.results[core]["out"]
    out += np.asarray(bo, np.float32)[None, :, None]
    if _trace:
        _cache["last_exec_ns"] = res.exec_time_ns
        _cache["last_res"] = res
    return out.reshape(B, C, D, HH, W)


# revision 12
# speedup vs baseline: 1.7354x; 1.7354x over previous
import sys
sys.path.insert(0, '/opt/trn_rl_repo')
import numpy as np
import concourse.bass as bass
import concourse.bacc as bacc
import concourse.mybir as mybir
from concourse.tile import TileContext
from concourse.bass_utils import run_bass_kernel_spmd

F32 = mybir.dt.float32
F32R = mybir.dt.float32r
BF16 = mybir.dt.bfloat16
AF = mybir.ActivationFunctionType
ALU = mybir.AluOpType
EPS = 1e-5

B, C, D, HH, W = 2, 256, 32, 32, 32
S = D * HH * W            # 32768 spatial per batch
NCORES = 8
SHARDS = 4                # spatial shards per batch
T_TOT = S // SHARDS       # 8192 tokens per core
CHUNK = 512
NCH = T_TOT // CHUNK      # 16 chunks
PIECE = 1024              # x load piece (cols)
NPIECE = T_TOT // PIECE   # 2
CTX, CD = 77, 768
H, HD = 8, 32
SCALE = HD ** -0.5
GN_N = float(32 * S)      # elements per group (32 ch x full spatial)

# packed param columns
PC_GG = 0      # 2 cols (per channel-half)
PC_GB = 2      # 2
PC_LG = 4      # 6
PC_LB = 10     # 6
PC_BD = 16     # 128 (32-block-diagonal ones)
PC_ID = 144    # 128 (identity)
PC_BM = 272    # 2 (batch-mask: col b = 1.0 if my batch == b)
NPAR = 274

_cache = {}


def _build():
    nc = bacc.Bacc("TRN2", target_bir_lowering=False, debug=False, num_devices=NCORES)
    dp = lambda n, s: nc.dram_tensor(n, s, F32, kind="ExternalInput").ap()
    x_d = dp("x", [C, T_TOT])
    ctx_d = dp("ctx", [CTX, CD])
    wkv_d = dp("wkv", [CD, 512])   # [Wk | Wv]
    wqo_d = dp("wqo", [C, 512])    # [Wq | Wo]
    par_d = dp("par", [128, NPAR])
    out_d = nc.dram_tensor("out", [C, T_TOT], F32, kind="ExternalOutput").ap()
    gn_in = nc.dram_tensor("gn_in", [128, 4], F32)
    gn_out = nc.dram_tensor("gn_out", [1024, 4], F32, addr_space="Shared")

    with TileContext(nc, num_cores=NCORES) as tc:
        with tc.tile_pool(name="const", bufs=1) as const, \
             tc.tile_pool(name="work", bufs=2) as work, \
             tc.tile_pool(name="pqt", bufs=1, space="PSUM") as p_qt, \
             tc.tile_pool(name="pscor", bufs=4, space="PSUM") as p_scor, \
             tc.tile_pool(name="pattn", bufs=1, space="PSUM") as p_attn, \
             tc.tile_pool(name="pzo", bufs=2, space="PSUM") as p_zo:

            # ---------- loads: x pieces first, then params/weights ----------
            xs = [[const.tile([128, PIECE], F32R, tag=f"x{i}{p}", name=f"x{i}{p}")
                   for p in range(NPIECE)] for i in range(2)]
            for p in range(NPIECE):
                for i in range(2):
                    nc.sync.dma_start(
                        out=xs[i][p][:],
                        in_=x_d[i*128:(i+1)*128, p*PIECE:(p+1)*PIECE].bitcast(F32R))
            par_sb = const.tile([128, NPAR], F32)
            nc.sync.dma_start(out=par_sb[:], in_=par_d[:])
            ctx_sb = const.tile([CTX, CD], F32)
            nc.sync.dma_start(out=ctx_sb[:], in_=ctx_d[:])
            wkv_sb = [const.tile([128, 512], F32, tag=f"wkv{k}", name=f"wkv{k}")
                      for k in range(6)]
            for k in range(6):
                nc.sync.dma_start(out=wkv_sb[k][:], in_=wkv_d[k*128:(k+1)*128, :])
            wqo_sb = [const.tile([128, 512], F32R, tag=f"wqo{i}", name=f"wqo{i}")
                      for i in range(2)]
            for i in range(2):
                nc.sync.dma_start(out=wqo_sb[i][:],
                                  in_=wqo_d[i*128:(i+1)*128, :].bitcast(F32R))

            # ---------- GroupNorm partial stats (pipelined with x arrival) ----------
            st = [work.tile([128, T_TOT//512, 6], F32, tag=f"st{i}", name=f"st{i}")
                  for i in range(2)]
            for p in range(NPIECE):
                for i in range(2):
                    for c in range(PIECE // 512):
                        j = p * (PIECE // 512) + c
                        nc.vector.bn_stats(out=st[i][:, j, :],
                                           in_=xs[i][p][:, c*512:(c+1)*512].bitcast(F32))
            partials = work.tile([128, 4], F32, tag="partials", name="partials")
            for i in range(2):
                mv = work.tile([128, 2], F32, tag="gnmv", name=f"gnmv{i}")
                nc.vector.bn_aggr(out=mv[:], in_=st[i][:])
                sq = work.tile([128, 1], F32, tag="gnsq", name=f"gnsq{i}")
                nc.vector.tensor_mul(sq[:], mv[:, 0:1], mv[:, 0:1])
                nc.vector.tensor_add(sq[:], sq[:], mv[:, 1:2])
                nc.vector.tensor_scalar_mul(partials[:, 2*i:2*i+1], mv[:, 0:1], float(T_TOT))
                nc.vector.tensor_scalar_mul(partials[:, 2*i+1:2*i+2], sq[:], float(T_TOT))
            nc.sync.dma_start(out=gn_in[:], in_=partials[:])
            nc.gpsimd.collective_compute(
                "AllGather", ALU.bypass,
                replica_groups=[[0, 1, 2, 3, 4, 5, 6, 7]],
                ins=[gn_in[:]], outs=[gn_out[:]])
            gw = work.tile([128, 8, 4], F32, tag="gw", name="gw")
            nc.sync.dma_start(out=gw[:],
                              in_=gn_out[:].rearrange("(b r) c -> r b c", b=8))
            u = work.tile([128, 2, 4], F32, tag="gu", name="gu")
            nc.vector.tensor_add(u[:], gw[:, 0:2, :], gw[:, 2:4, :])   # batch0 blocks
            v = work.tile([128, 2, 4], F32, tag="gv", name="gv")
            nc.vector.tensor_add(v[:], gw[:, 4:6, :], gw[:, 6:8, :])   # batch1 blocks
            nc.vector.tensor_add(u[:, 0, :], u[:, 0, :], u[:, 1, :])
            nc.vector.tensor_add(v[:, 0, :], v[:, 0, :], v[:, 1, :])
            nc.vector.tensor_scalar_mul(u[:, 0, :], u[:, 0, :], par_sb[:, PC_BM:PC_BM+1])
            nc.vector.tensor_scalar_mul(v[:, 0, :], v[:, 0, :], par_sb[:, PC_BM+1:PC_BM+2])
            nc.vector.tensor_add(u[:, 0, :], u[:, 0, :], v[:, 0, :])
            gps = p_zo.tile([128, 4], F32, tag="zo", name="gps")
            nc.tensor.matmul(gps[:], par_sb[:, PC_BD:PC_BD+128], u[:, 0, :],
                             start=True, stop=True)

            eps_t = work.tile([128, 1], F32, tag="eps", name="eps")
            nc.vector.memset(eps_t[:], EPS)
            a_sb, b_sb = [], []
            for i in range(2):
                mu = work.tile([128, 1], F32, tag=f"mu{i}", name=f"mu{i}")
                nc.vector.tensor_scalar_mul(mu[:], gps[:, 2*i:2*i+1], 1.0 / GN_N)
                ms = work.tile([128, 1], F32, tag=f"ms{i}", name=f"ms{i}")
                nc.vector.tensor_scalar_mul(ms[:], gps[:, 2*i+1:2*i+2], 1.0 / GN_N)
                var = work.tile([128, 1], F32, tag=f"var{i}", name=f"var{i}")
                nc.vector.tensor_mul(var[:], mu[:], mu[:])
                nc.vector.tensor_sub(var[:], ms[:], var[:])
                std = work.tile([128, 1], F32, tag=f"std{i}", name=f"std{i}")
                nc.scalar.activation(out=std[:], in_=var[:], func=AF.Sqrt, bias=eps_t[:])
                rstd = work.tile([128, 1], F32, tag=f"rstd{i}", name=f"rstd{i}")
                nc.vector.reciprocal(out=rstd[:], in_=std[:])
                ai = const.tile([128, 1], F32, tag=f"ai{i}", name=f"ai{i}")
                nc.vector.tensor_mul(ai[:], rstd[:], par_sb[:, PC_GG+i:PC_GG+i+1])
                bi = const.tile([128, 1], F32, tag=f"bi{i}", name=f"bi{i}")
                nc.vector.tensor_mul(bi[:], mu[:], ai[:])
                nc.vector.tensor_sub(bi[:], par_sb[:, PC_GB+i:PC_GB+i+1], bi[:])
                a_sb.append(ai)
                b_sb.append(bi)

            # ---------- context layernorm + K/V prep (overlaps x/stats) ----------
            stats = work.tile([CTX, 3, 6], F32, tag="lnstats", name="lnstats")
            for i in range(3):
                nc.vector.bn_stats(out=stats[:, i, :], in_=ctx_sb[:, i*256:(i+1)*256])
            mvc = work.tile([CTX, 2], F32, tag="lnmv", name="lnmv")
            nc.vector.bn_aggr(out=mvc[:], in_=stats[:])
            stdc = work.tile([CTX, 1], F32, tag="lnstd", name="lnstd")
            nc.scalar.activation(out=stdc[:], in_=mvc[:, 1:2], func=AF.Sqrt,
                                 bias=eps_t[:CTX, :])
            rstdc = work.tile([CTX, 1], F32, tag="lnrstd", name="lnrstd")
            nc.vector.reciprocal(out=rstdc[:], in_=stdc[:])
            ctxn = work.tile([CTX, CD], F32, tag="ctxn", name="ctxn")
            nc.vector.tensor_scalar(out=ctxn[:], in0=ctx_sb[:], scalar1=mvc[:, 0:1],
                                    scalar2=rstdc[:], op0=ALU.subtract, op1=ALU.mult)

            # transpose ctxn -> 6 tiles [128, 77] bf16, fold ln gamma/beta
            ctxnT = [const.tile([128, CTX], F32, tag=f"cT{k}", name=f"cT{k}")
                     for k in range(6)]
            for k in range(6):
                ps = p_zo.tile([128, CTX], F32, tag="zo", name=f"psT{k}")
                nc.tensor.transpose(ps[:, :], ctxn[:, k*128:(k+1)*128],
                                    par_sb[:CTX, PC_ID:PC_ID+CTX])
                nc.vector.tensor_scalar(out=ctxnT[k][:], in0=ps[:],
                                        scalar1=par_sb[:, PC_LG+k:PC_LG+k+1],
                                        scalar2=par_sb[:, PC_LB+k:PC_LB+k+1],
                                        op0=ALU.mult, op1=ALU.add)

            # kT [2 x (128, 77)] scaled by 1/sqrt(hd)
            kT = [const.tile([128, CTX], F32R, tag=f"kT{m}", name=f"kT{m}")
                  for m in range(2)]
            for m in range(2):
                ps = p_zo.tile([128, CTX], F32, tag="zo", name=f"psK{m}")
                for k in range(6):
                    nc.tensor.matmul(ps[:], wkv_sb[k][:, m*128:(m+1)*128],
                                     ctxnT[k][:], start=(k == 0), stop=(k == 5))
                nc.vector.tensor_scalar_mul(kT[m][:], ps[:], SCALE)

            # v [77, 256]
            v_sb = const.tile([CTX, C], F32)
            psv = p_attn.tile([CTX, C], F32, tag="attn", name="psV")
            for k in range(6):
                nc.tensor.matmul(psv[:], ctxnT[k][:], wkv_sb[k][:, 256:512],
                                 start=(k == 0), stop=(k == 5))
            nc.vector.tensor_copy(v_sb[:], psv[:])

            # zero-padded V / ones lhsT tiles [77, 128]
            zpad = const.tile([CTX, 128], F32)
            nc.gpsimd.memset(zpad[:], 0.0)
            onescol = const.tile([CTX, 32], F32)
            nc.gpsimd.memset(onescol[:], 1.0)
            vpad, opad = [], []
            for g in range(2):
                vp = [const.tile([CTX, 128], F32R, tag=f"vp{g}{j}", name=f"vp{g}{j}")
                      for j in range(4)]
                op = [const.tile([CTX, 128], F32R, tag=f"op{g}{j}", name=f"op{g}{j}")
                      for j in range(4)]
                for j in range(4):
                    h = 4*g + j
                    nc.gpsimd.tensor_copy(vp[j][:], zpad[:])
                    nc.gpsimd.tensor_copy(vp[j][:, 32*j:32*(j+1)], v_sb[:, 32*h:32*(h+1)])
                    nc.gpsimd.tensor_copy(op[j][:], zpad[:])
                    nc.gpsimd.tensor_copy(op[j][:, 32*j:32*(j+1)], onescol[:])
                vpad.append(vp)
                opad.append(op)

            # ---------- main token loop (software-pipelined) ----------
            state = {}  # per-(t-1) tiles carried across pipeline stages

            def emit_A(t):
                p, off = t // (PIECE // CHUNK), (t % (PIECE // CHUNK)) * CHUNK
                xn = []
                for i in range(2):
                    xni = work.tile([128, CHUNK], F32R, tag=f"xn{i}", name=f"xn{i}_{t}")
                    nc.gpsimd.tensor_scalar(out=xni[:],
                                            in0=xs[i][p][:, off:off+CHUNK].bitcast(F32),
                                            scalar1=a_sb[i][:], scalar2=b_sb[i][:],
                                            op0=ALU.mult, op1=ALU.add)
                    xn.append(xni)
                return {"t": t, "xn": xn, "p": p, "off": off}

            def emit_qt(A, m):
                ps = p_qt.tile([128, CHUNK], F32, tag="qt", name=f"psqt{m}_{A['t']}")
                for i in range(2):
                    nc.tensor.matmul(ps[:], wqo_sb[i][:, m*128:(m+1)*128],
                                     A["xn"][i][:], start=(i == 0), stop=(i == 1))
                qm = work.tile([128, CHUNK], F32R, tag=f"qT{m}", name=f"qT{m}_{A['t']}")
                nc.vector.tensor_copy(qm[:], ps[:])
                return qm

            def emit_scores(A, g, js, qm):
                t = A["t"]
                if "exps" not in A:
                    A["exps"] = {}
                if g not in A["exps"]:
                    A["exps"][g] = work.tile([CTX, 4*CHUNK], F32R, tag=f"exps{g}",
                                             name=f"exps{g}_{t}")
                ex = A["exps"][g]
                for j in js:
                    sl = p_scor.tile([CTX, CHUNK], F32, tag="scor", name=f"sc{g}{j}_{t}")
                    nc.tensor.matmul(sl[:], kT[g][32*j:32*(j+1), :],
                                     qm[32*j:32*(j+1), :],
                                     start=True, stop=True, tile_position=(32*j, 0))
                    nc.scalar.activation(out=ex[:, j*CHUNK:(j+1)*CHUNK], in_=sl[:],
                                         func=AF.Exp)

            def emit_attn(Bst, g):
                t = Bst["t"]
                ap = p_attn.tile([128, CHUNK], F32, tag="attn", name=f"attn{g}_{t}")
                for j in range(4):
                    nc.tensor.matmul(ap[:], vpad[g][j][:],
                                     Bst["exps"][g][:, j*CHUNK:(j+1)*CHUNK],
                                     start=(j == 0), stop=(j == 3))
                Bst[f"attn{g}"] = ap

            def emit_zb(Bst, g):
                t = Bst["t"]
                zp = p_zo.tile([128, CHUNK], F32, tag="zo", name=f"zb{g}_{t}")
                for j in range(4):
                    nc.tensor.matmul(zp[:], opad[g][j][:],
                                     Bst["exps"][g][:, j*CHUNK:(j+1)*CHUNK],
                                     start=(j == 0), stop=(j == 3))
                zr = work.tile([128, CHUNK], F32, tag=f"zr{g}", name=f"zr{g}_{t}")
                nc.vector.reciprocal(out=zr[:], in_=zp[:])
                Bst[f"zr{g}"] = zr

            def emit_anrm(Bst, g):
                an = work.tile([128, CHUNK], F32R, tag=f"anrm{g}", name=f"anrm{g}_{Bst['t']}")
                nc.vector.tensor_mul(an[:], Bst[f"attn{g}"][:], Bst[f"zr{g}"][:])
                Bst[f"anrm{g}"] = an

            def emit_out(Bst, m):
                t = Bst["t"]
                ps = p_zo.tile([128, CHUNK], F32, tag="zo", name=f"psout{m}_{t}")
                for g in range(2):
                    nc.tensor.matmul(ps[:], wqo_sb[g][:, 256+m*128:256+(m+1)*128],
                                     Bst[f"anrm{g}"][:], start=(g == 0), stop=(g == 1))
                o = work.tile([128, CHUNK], F32, tag=f"osb{m}", name=f"osb{m}_{t}")
                p, off = Bst["p"], Bst["off"]
                nc.vector.tensor_add(o[:], ps[:], xs[m][p][:, off:off+CHUNK].bitcast(F32))
                t0 = t * CHUNK
                nc.sync.dma_start(out=out_d[m*128:(m+1)*128, t0:t0+CHUNK], in_=o[:])

            prev = None
            for t in range(NCH):
                A = emit_A(t)
                q0m = emit_qt(A, 0)
                if prev is not None:
                    emit_attn(prev, 0)
                q1m = emit_qt(A, 1)
                if prev is not None:
                    emit_zb(prev, 0)
                emit_scores(A, 0, [0, 1], q0m)
                if prev is not None:
                    emit_anrm(prev, 0)
                    emit_attn(prev, 1)
                emit_scores(A, 0, [2, 3], q0m)
                if prev is not None:
                    emit_zb(prev, 1)
                emit_scores(A, 1, [0, 1, 2, 3], q1m)
                if prev is not None:
                    emit_anrm(prev, 1)
                    emit_out(prev, 0)
                    emit_out(prev, 1)
                prev = A
            # drain last chunk
            emit_attn(prev, 0)
            emit_zb(prev, 0)
            emit_anrm(prev, 0)
            emit_attn(prev, 1)
            emit_zb(prev, 1)
            emit_anrm(prev, 1)
            emit_out(prev, 0)
            emit_out(prev, 1)

    nc.compile()
    return nc


def _get_nc():
    if "nc" not in _cache:
        _cache["nc"] = _build()
    return _cache["nc"]


def kernel(x, context, gn_gamma, gn_beta, ln_gamma, ln_beta, Wq, Wk, Wv, Wo, bo,
           _trace=False):
    nc = _get_nc()
    x = np.asarray(x, dtype=np.float32)
    xr = x.reshape(B, C, S)
    par = np.zeros((128, NPAR), np.float32)
    par[:, PC_GG:PC_GG+2] = np.asarray(gn_gamma, np.float32).reshape(2, 128).T
    par[:, PC_GB:PC_GB+2] = np.asarray(gn_beta, np.float32).reshape(2, 128).T
    par[:, PC_LG:PC_LG+6] = np.asarray(ln_gamma, np.float32).reshape(6, 128).T
    par[:, PC_LB:PC_LB+6] = np.asarray(ln_beta, np.float32).reshape(6, 128).T
    par[:, PC_BD:PC_BD+128] = np.kron(np.eye(4, dtype=np.float32),
                                      np.ones((32, 32), np.float32))
    par[:, PC_ID:PC_ID+128] = np.eye(128, dtype=np.float32)
    common = {
        "wkv": np.ascontiguousarray(np.concatenate(
            [np.asarray(Wk, np.float32), np.asarray(Wv, np.float32)], axis=1)),
        "wqo": np.ascontiguousarray(np.concatenate(
            [np.asarray(Wq, np.float32), np.asarray(Wo, np.float32)], axis=1)),
        "par": par,
    }
    in_maps = []
    for core in range(NCORES):
        b, s = core // SHARDS, core % SHARDS
        m = dict(common)
        pc = par.copy()
        pc[:, PC_BM + b] = 1.0
        m["par"] = pc
        m["x"] = np.ascontiguousarray(xr[b][:, s*T_TOT:(s+1)*T_TOT])
        m["ctx"] = np.ascontiguousarray(np.asarray(context, np.float32)[b])
        in_maps.append(m)
    res = run_bass_kernel_spmd(nc, in_maps, list(range(NCORES)), trace=_trace)
    out = np.empty((B, C, S), np.float32)
    for core in range(NCORES):
        b, s = core // SHARDS, core % SHARDS
        out[b][:, s*T_TOT:(s+1)*T_TOT] = res.results[core]["out"]
    out += np.asarray(bo, np.float32)[None, :, None]
    if _trace:
        _cache["last_exec_ns"] = res.exec_time_ns
        _cache["last_res"] = res
    return out.reshape(B, C, D, HH, W)
